# revision 5
# baseline (speedup 1.0000x reference)
"""Depthwise causal Conv1d (k=4) + SiLU on 8 Trainium2 NeuronCores.

Problem: x [4, 4096, 2048] f32, w [2048, 4] f32,
out[b, t, d] = silu(sum_j w[d, j] * x[b, t - 3 + j, d])   (zero-padded left).

Sharding: 8 cores = 4 batches x 2 channel-halves. Depthwise conv is
independent per channel, so channel sharding needs no halo exchange.

Layout: each core receives its shard host-transposed to [channels, time]
(channels on SBUF partitions). The per-channel weight w[d, j] is then a
per-partition scalar, and the causal time shifts are free-dim AP offsets
into one loaded tile. The output DRAM layout is half-major
[2*DH, 2048] so every [128, 2048] store is fully dense (pitch == width).

Precision: x and the output are host-cast fp16 (halves HBM traffic both
ways); products and adds stay fp16 (PE accumulates fp32 in PSUM); SiLU
computes fp32-internally on ACT. End-to-end relative error ~5e-4.

v4 design, tuned against NTFF profiles of v1-v3:
 - Per-core budget is DMA: ~16.9 MB at ~435 GB/s (R+W combined) ~ 40us.
 - TensorEngine p-states reach 2.4 GHz only after ~3us of CONTINUOUS
   execution, so all PE work (5 of 8 blocks, diag(w_j) matmuls, 4 taps
   PSUM-accumulated) is one back-to-back stream; the emission order of
   ACT ops is hand-matched to production order so a PSUM drain never
   heads-of-line-blocks the PE. Taps iterate OUTER (same stationary for
   4 consecutive matmuls) to enable weight-load reuse.
 - Diag stationaries are built on device from a [128,128] identity mask
   (20 cheap DVE ops) instead of a 1 MB HBM tensor (v1).
 - 3 blocks ride DVE full-length: 4 shift-rebased tensor_scalar
   products + pair-packed adds (the v2 scalar_tensor_tensor MAC chain
   measured 2x slower per column and was reverted).
 - ACT does only SiLU. Loads are full padded rows on SyncE (HWDGE);
   stores are dense [128,2048] chunks on GpSimd (SWDGE).
"""

import sys
import types

import numpy as np

import concourse.bass as bass
import concourse.bacc as bacc
import concourse.mybir as mybir
from concourse.tile import TileContext
from concourse.bass_utils import run_bass_kernel_spmd


def _ensure_ntff_hook():
    """bass_utils imports antenv.axon_hooks when BASS_TRACE is set; that
    module is absent on this image. Install a shim so tracing works when
    possible and degrades gracefully (instead of crashing) when not."""
    try:
        import antenv.axon_hooks  # noqa: F401

        return
    except ImportError:
        pass
    try:
        import antenv

        hook = None
        try:
            if "/root/.axon_site" not in sys.path:
                sys.path.insert(0, "/root/.axon_site")
            from trn_agent_boot.trn_boot import _ntff_profile_via_ctypes

            hook = _ntff_profile_via_ctypes("/opt/axon/libaxon_pjrt.so")
        except Exception:
            hook = None
        mod = types.ModuleType("antenv.axon_hooks")
        mod._hook = hook
        mod.get_axon_ntff_profile_hook = lambda: mod._hook
        mod.set_axon_ntff_profile_hook = lambda h: setattr(mod, "_hook", h)
        sys.modules["antenv.axon_hooks"] = mod
        antenv.axon_hooks = mod
    except Exception:
        pass


_ensure_ntff_hook()

B, L, D = 4, 4096, 2048
K = 4
PAD = K - 1
N_CORES = 8
DH = D // 2            # channels per core
NBLK = DH // 128       # 128-partition channel blocks per core
ROWW = 4128            # DRAM row stride (fp16 elems): 64B-aligned rows
C = 2048               # time chunk (half of L)

MID_DT = mybir.dt.float16
PE_BLKS = [1, 3, 5, 7, 6]   # blocks on the TensorEngine, in stream order
DVE_BLKS = [0, 2, 4]        # blocks on DVE
_PE_IDX = {b: i for i, b in enumerate(PE_BLKS)}

_cache = {}


def _build_bass():
    nc = bacc.Bacc()
    xt = nc.dram_tensor("xt", [DH, ROWW], MID_DT, kind="ExternalInput")
    wt = nc.dram_tensor("wt", [128, NBLK * K], mybir.dt.float32, kind="ExternalInput")
    # [128,128] identity mask; diag(w_j) stationaries are built on device
    dg = nc.dram_tensor("dg", [128, 128], MID_DT, kind="ExternalInput")
    # half-major output: row (half*DH + ch), col t-in-half; stores are dense
    ot = nc.dram_tensor("ot", [2 * DH, C], MID_DT, kind="ExternalOutput")
    f32 = mybir.dt.float32

    with TileContext(nc) as tc:
        with tc.tile_pool(name="pool", bufs=2) as pool, \
             tc.tile_pool(name="psum", bufs=2, space="PSUM") as psum_pool:
            # Warmup: a tiny Silu forces the silu activation-table set to
            # load during the initial DMA wait; it is the only table load
            # in the whole kernel.
            warm = pool.tile([128, 2], MID_DT, tag="warm", bufs=1)
            nc.vector.memset(warm[:], 0.0)
            nc.scalar.activation(warm[:], warm[:], mybir.ActivationFunctionType.Silu)

            w = pool.tile([128, NBLK * K], f32, tag="w", bufs=1)
            nc.sync.dma_start(out=w[:], in_=wt[:, :])
            mask = pool.tile([128, 128], MID_DT, tag="mask", bufs=1)
            nc.sync.dma_start(out=mask[:], in_=dg[:, :])

            # Full padded-row loads, PE blocks interleaved first so the
            # TensorEngine stream starts as early as possible.
            xts = {}
            for blk in [1, 0, 3, 2, 5, 4, 7, 6]:
                r0 = blk * 128
                x = pool.tile([128, L + PAD + 1], MID_DT, tag=f"x{blk}", bufs=1)
                nc.sync.dma_start(
                    out=x[:, 0 : L + PAD], in_=xt[r0 : r0 + 128, 0 : L + PAD]
                )
                xts[blk] = x

            # diag(w[blk*128+p, j]) stationaries: [128,128] per-partition
            # scalar muls of the identity mask, first PE block first.
            dgw = pool.tile([128, len(PE_BLKS) * K * 128], MID_DT, tag="dgw", bufs=1)
            for blk in PE_BLKS:
                bi = _PE_IDX[blk]
                for j in range(K):
                    c0 = (bi * K + j) * 128
                    nc.vector.tensor_scalar_mul(
                        dgw[:, c0 : c0 + 128],
                        mask[:],
                        w[:, blk * K + j : blk * K + j + 1],
                    )

            def wj(blk, j):
                return w[:, blk * K + j : blk * K + j + 1]

            ps_of = {}

            def pe_mm(blk, half):
                """Fill one [128, 2048] PSUM tile with the 4-tap conv of
                one half of a PE block. Taps outer: one stationary feeds
                4 consecutive matmuls."""
                bi = _PE_IDX[blk]
                x = xts[blk]
                t0 = half * C
                ps = psum_pool.tile([128, C], f32, tag="ps", bufs=2)
                ps_of[(blk, half)] = ps
                for j in range(K):
                    lw = dgw[:, (bi * K + j) * 128 : (bi * K + j + 1) * 128]
                    for c in range(C // 512):
                        nc.tensor.matmul(
                            ps[:, c * 512 : (c + 1) * 512],
                            lw,
                            x[:, t0 + c * 512 + j : t0 + c * 512 + j + 512],
                            start=(j == 0),
                            stop=(j == K - 1),
                        )

            def pe_fin(blk, half):
                """SiLU straight out of PSUM, then dense store."""
                r0 = blk * 128
                ps = ps_of.pop((blk, half))
                o = pool.tile([128, C], MID_DT, tag="o", bufs=6)
                nc.scalar.activation(o[:], ps[:], mybir.ActivationFunctionType.Silu)
                nc.gpsimd.dma_start(
                    out=ot[half * DH + r0 : half * DH + r0 + 128, :], in_=o[:]
                )

            qe_of = {}

            def dve_chain(blk):
                """Full-length elementwise conv: 4 shift-rebased products,
                pair-packed adds (qe=[q0|q2] + qo=[q1|q3], then fold)."""
                x = xts[blk]
                qe = pool.tile([128, 2, L], MID_DT, tag="qe", bufs=2)
                qo = pool.tile([128, 2, L], MID_DT, tag="qo", bufs=2)
                qe_of[blk] = qe
                nc.vector.tensor_scalar_mul(qe[:, 0, :], x[:, 0:L], wj(blk, 0))
                nc.vector.tensor_scalar_mul(qo[:, 0, :], x[:, 1 : 1 + L], wj(blk, 1))
                nc.vector.tensor_scalar_mul(qe[:, 1, :], x[:, 2 : 2 + L], wj(blk, 2))
                nc.vector.tensor_scalar_mul(qo[:, 1, :], x[:, 3 : 3 + L], wj(blk, 3))
                nc.vector.tensor_add(qe[:, :, :], qe[:, :, :], qo[:, :, :])
                nc.vector.tensor_add(qe[:, 0, :], qe[:, 0, :], qe[:, 1, :])

            def dve_fin(blk, half):
                r0 = blk * 128
                qe = qe_of[blk]
                o = pool.tile([128, C], MID_DT, tag="o", bufs=6)
                nc.scalar.activation(
                    o[:], qe[:, 0, half * C : half * C + C],
                    mybir.ActivationFunctionType.Silu,
                )
                nc.gpsimd.dma_start(
                    out=ot[half * DH + r0 : half * DH + r0 + 128, :], in_=o[:]
                )

            # TensorEngine stream: back-to-back, no interleaving.
            for blk in PE_BLKS:
                pe_mm(blk, 0)
                pe_mm(blk, 1)

            # DVE stream: the three elementwise blocks in sequence.
            for blk in DVE_BLKS:
                dve_chain(blk)

            # ACT + store stream, emission matched to production order so
            # the in-order ACT engine never blocks a ready PSUM drain
            # behind a not-yet-ready DVE silu (or vice versa).
            for blk, half in [
                (1, 0), (1, 1), (3, 0), (0, 0), (3, 1), (0, 1),
                (5, 0), (2, 0), (5, 1), (2, 1),
                (7, 0), (4, 0), (7, 1), (4, 1),
                (6, 0), (6, 1),
            ]:
                if blk in _PE_IDX:
                    pe_fin(blk, half)
                else:
                    dve_fin(blk, half)
    nc.compile()
    return nc


def _shard_inputs(x, w):
    in_maps = []
    dg = np.eye(128, dtype=np.float16)
    for core in range(N_CORES):
        b, half = divmod(core, 2)
        d0 = half * DH
        xt = np.zeros((DH, ROWW), dtype=np.float16)
        xt[:, PAD : PAD + L] = x[b, :, d0 : d0 + DH].T.astype(np.float16)
        # w rows for this shard, rearranged so partition p holds the K
        # weights of channel blk*128 + p at free cols [blk*K, blk*K + K)
        w_sh = w[d0 : d0 + DH].reshape(NBLK, 128, K)
        wt = (
            w_sh.transpose(1, 0, 2).reshape(128, NBLK * K).astype(np.float32)
        )
        in_maps.append(
            {
                "xt": np.ascontiguousarray(xt),
                "wt": np.ascontiguousarray(wt),
                "dg": dg,
            }
        )
    return in_maps


def kernel(x, w):
    x = np.asarray(x, dtype=np.float32)
    w = np.asarray(w, dtype=np.float32)
    assert x.shape == (B, L, D) and w.shape == (D, K)

    if "nc" not in _cache:
        _cache["nc"] = _build_bass()
    nc = _cache["nc"]

    in_maps = _shard_inputs(x, w)
    res = None
    for attempt in range(3):
        try:
            res = run_bass_kernel_spmd(nc, in_maps, core_ids=list(range(N_CORES)))
            break
        except Exception:
            if attempt == 2:
                raise
    _cache["last_results"] = res

    out = np.empty((B, L, D), dtype=np.float32)
    for core in range(N_CORES):
        b, half = divmod(core, 2)
        d0 = half * DH
        o3 = res.results[core]["ot"].reshape(2, DH, C)
        full = np.concatenate([o3[0], o3[1]], axis=1)  # [DH, L]
        out[b, :, d0 : d0 + DH] = full.T.astype(np.float32)
    return out


# revision 6
# speedup vs baseline: 1.1216x; 1.1216x over previous
"""Depthwise causal Conv1d (k=4) + SiLU on 8 Trainium2 NeuronCores.

Problem: x [4, 4096, 2048] f32, w [2048, 4] f32,
out[b, t, d] = silu(sum_j w[d, j] * x[b, t - 3 + j, d])   (zero-padded left).

Sharding: 8 cores = 4 batches x 2 channel-halves. Depthwise conv is
independent per channel, so channel sharding needs no halo exchange.

Layout: each core receives its shard host-transposed to [channels, time]
(channels on SBUF partitions); per-channel weights are per-partition
scalars and causal shifts are free-dim AP offsets. Both DRAM tensors are
HALF-MAJOR so every [128, ~2048] DMA row is dense: xt row (half*DH+ch)
holds x[ch, half*2048-3 : half*2048+2048] (3-col halo duplicated on the
host), ot row (half*DH+ch) holds out[ch, half*2048 : +2048].

Precision: x and the output are host-cast fp16 (halves HBM traffic both
ways); products and adds stay fp16 (PE accumulates fp32 in PSUM); SiLU
computes fp32-internally on ACT. End-to-end relative error ~5e-4.

v5 design, tuned against NTFF profiles of v1-v4:
 - Per-core budget is DMA: ~16.9 MB at ~435 GB/s (R+W combined) ~ 40us;
   dense rows keep both directions near the fabric cap.
 - Work is cut into [128ch, 2048t] chunks. 5 blocks ride the
   TensorEngine as one back-to-back stream (p-states only reach 2.4 GHz
   under continuous execution): diag(w_j) matmuls, taps outer, 4-tap
   PSUM accumulation, SiLU straight out of PSUM. 3 blocks ride DVE:
   4 shift-rebased tensor_scalar products + pair-packed adds.
 - Diag stationaries are built on device from a [128,128] identity mask
   (20 cheap DVE ops) instead of a 1 MB HBM tensor.
 - ACT does only SiLU; its emission order is hand-matched to production
   order so the in-order engine never blocks a ready PSUM drain behind
   a not-yet-ready DVE silu. Stores follow silus chunk by chunk, so the
   store stream starts ~5us in and overlaps the load stream.
 - Loads on SyncE (HWDGE, 8-chunk runway), stores on GpSimd (SWDGE).
"""

import sys
import types

import numpy as np

import concourse.bass as bass
import concourse.bacc as bacc
import concourse.mybir as mybir
from concourse.tile import TileContext
from concourse.bass_utils import run_bass_kernel_spmd


def _ensure_ntff_hook():
    """bass_utils imports antenv.axon_hooks when BASS_TRACE is set; that
    module is absent on this image. Install a shim so tracing works when
    possible and degrades gracefully (instead of crashing) when not."""
    try:
        import antenv.axon_hooks  # noqa: F401

        return
    except ImportError:
        pass
    try:
        import antenv

        hook = None
        try:
            if "/root/.axon_site" not in sys.path:
                sys.path.insert(0, "/root/.axon_site")
            from trn_agent_boot.trn_boot import _ntff_profile_via_ctypes

            hook = _ntff_profile_via_ctypes("/opt/axon/libaxon_pjrt.so")
        except Exception:
            hook = None
        mod = types.ModuleType("antenv.axon_hooks")
        mod._hook = hook
        mod.get_axon_ntff_profile_hook = lambda: mod._hook
        mod.set_axon_ntff_profile_hook = lambda h: setattr(mod, "_hook", h)
        sys.modules["antenv.axon_hooks"] = mod
        antenv.axon_hooks = mod
    except Exception:
        pass


_ensure_ntff_hook()

B, L, D = 4, 4096, 2048
K = 4
PAD = K - 1
N_CORES = 8
DH = D // 2            # channels per core
NBLK = DH // 128       # 128-partition channel blocks per core
C = 2048               # time chunk (half of L)
XROW = C + PAD         # 2051 data cols per xt row
XPITCH = 2064          # xt row pitch (fp16 elems), 32B-aligned

MID_DT = mybir.dt.float16
PE_BLKS = [1, 3, 5, 7, 6]   # blocks on the TensorEngine, in stream order
DVE_BLKS = [0, 2, 4]        # blocks on DVE
_PE_IDX = {b: i for i, b in enumerate(PE_BLKS)}

_cache = {}


def _build_bass():
    nc = bacc.Bacc()
    # half-major inputs/outputs: row (half*DH + ch)
    xt = nc.dram_tensor("xt", [2 * DH, XPITCH], MID_DT, kind="ExternalInput")
    wt = nc.dram_tensor("wt", [128, NBLK * K], mybir.dt.float32, kind="ExternalInput")
    # [128,128] identity mask; diag(w_j) stationaries are built on device
    dg = nc.dram_tensor("dg", [128, 128], MID_DT, kind="ExternalInput")
    ot = nc.dram_tensor("ot", [2 * DH, C], MID_DT, kind="ExternalOutput")
    f32 = mybir.dt.float32

    with TileContext(nc) as tc:
        with tc.tile_pool(name="pool", bufs=2) as pool, \
             tc.tile_pool(name="psum", bufs=2, space="PSUM") as psum_pool:
            # Warmup: a tiny Silu forces the silu activation-table set to
            # load during the initial DMA wait; it is the only table load
            # in the whole kernel.
            warm = pool.tile([128, 2], MID_DT, tag="warm", bufs=1)
            nc.vector.memset(warm[:], 0.0)
            nc.scalar.activation(warm[:], warm[:], mybir.ActivationFunctionType.Silu)

            w = pool.tile([128, NBLK * K], f32, tag="w", bufs=1)
            nc.sync.dma_start(out=w[:], in_=wt[:, :])
            mask = pool.tile([128, 128], MID_DT, tag="mask", bufs=1)
            nc.sync.dma_start(out=mask[:], in_=dg[:, :])

            # diag(w[blk*128+p, j]) stationaries: [128,128] per-partition
            # scalar muls of the identity mask, first PE block first.
            dgw = pool.tile([128, len(PE_BLKS) * K * 128], MID_DT, tag="dgw", bufs=1)
            for blk in PE_BLKS:
                bi = _PE_IDX[blk]
                for j in range(K):
                    c0 = (bi * K + j) * 128
                    nc.vector.tensor_scalar_mul(
                        dgw[:, c0 : c0 + 128],
                        mask[:],
                        w[:, blk * K + j : blk * K + j + 1],
                    )

            def wj(blk, j):
                return w[:, blk * K + j : blk * K + j + 1]

            xts = {}

            def load(blk, half):
                r0 = half * DH + blk * 128
                x = pool.tile([128, XROW + 1], MID_DT, tag="x", bufs=8)
                nc.sync.dma_start(out=x[:, 0:XROW], in_=xt[r0 : r0 + 128, 0:XROW])
                xts[(blk, half)] = x

            ps_of = {}

            def pe_mm(blk, half):
                """Fill one [128, 2048] PSUM tile with the 4-tap conv of
                one chunk. Taps outer: one stationary per 4 matmuls."""
                bi = _PE_IDX[blk]
                x = xts[(blk, half)]
                ps = psum_pool.tile([128, C], f32, tag="ps", bufs=2)
                ps_of[(blk, half)] = ps
                for j in range(K):
                    lw = dgw[:, (bi * K + j) * 128 : (bi * K + j + 1) * 128]
                    for c in range(C // 512):
                        nc.tensor.matmul(
                            ps[:, c * 512 : (c + 1) * 512],
                            lw,
                            x[:, c * 512 + j : c * 512 + j + 512],
                            start=(j == 0),
                            stop=(j == K - 1),
                        )

            qe_of = {}

            def dve_chain(blk, half):
                """Elementwise chunk: 4 shift-rebased products, pair-packed
                adds (qe=[q0|q2] + qo=[q1|q3], then fold into qe0)."""
                x = xts[(blk, half)]
                qe = pool.tile([128, 2, C], MID_DT, tag="qe", bufs=2)
                qo = pool.tile([128, 2, C], MID_DT, tag="qo", bufs=2)
                qe_of[(blk, half)] = qe
                nc.vector.tensor_scalar_mul(qe[:, 0, :], x[:, 0:C], wj(blk, 0))
                nc.vector.tensor_scalar_mul(qo[:, 0, :], x[:, 1 : 1 + C], wj(blk, 1))
                nc.vector.tensor_scalar_mul(qe[:, 1, :], x[:, 2 : 2 + C], wj(blk, 2))
                nc.vector.tensor_scalar_mul(qo[:, 1, :], x[:, 3 : 3 + C], wj(blk, 3))
                nc.vector.tensor_add(qe[:, :, :], qe[:, :, :], qo[:, :, :])
                nc.vector.tensor_add(qe[:, 0, :], qe[:, 0, :], qe[:, 1, :])

            def fin(blk, half):
                """SiLU (PSUM for PE chunks, SBUF for DVE chunks) + dense
                store of the finished [128, 2048] chunk."""
                r0 = half * DH + blk * 128
                o = pool.tile([128, C], MID_DT, tag="o", bufs=6)
                if (blk, half) in ps_of:
                    src = ps_of.pop((blk, half))[:]
                else:
                    src = qe_of.pop((blk, half))[:, 0, :]
                nc.scalar.activation(o[:], src, mybir.ActivationFunctionType.Silu)
                nc.gpsimd.dma_start(out=ot[r0 : r0 + 128, :], in_=o[:])

            # Chunk schedule. P = TensorEngine chunks in stream order,
            # E = DVE chunks. Loads are emitted in consumption order (the
            # bufs=8 x-pool gives the HWDGE queue an 8-chunk runway);
            # compute is emitted per-engine back-to-back; ACT/store order
            # approximates production order.
            P = [(b, h) for h in (0, 1) for b in PE_BLKS]
            E = [(b, h) for h in (0, 1) for b in DVE_BLKS]
            load_order = [
                P[0], E[0], P[1], E[1], P[2], E[2], P[3], E[3],
                P[4], E[4], P[5], E[5], P[6], P[7], P[8], P[9],
            ]
            for blk, half in load_order:
                load(blk, half)

            # Per-engine compute streams (Tile resolves cross-engine deps;
            # program order only pins intra-engine order).
            for blk, half in P:
                pe_mm(blk, half)
            for blk, half in E:
                dve_chain(blk, half)

            # ACT + store order matched to expected completion times:
            # PE chunk i completes ~4us apart; DVE chunk j ~6.4us apart.
            fin_order = [
                P[0], P[1], E[0], P[2], E[1], P[3], P[4], E[2],
                P[5], E[3], P[6], P[7], E[4], P[8], E[5], P[9],
            ]
            for blk, half in fin_order:
                fin(blk, half)
    nc.compile()
    return nc


def _shard_inputs(x, w):
    in_maps = []
    dg = np.eye(128, dtype=np.float16)
    for core in range(N_CORES):
        b, half = divmod(core, 2)
        d0 = half * DH
        xp = np.zeros((DH, PAD + L), dtype=np.float16)
        xp[:, PAD:] = x[b, :, d0 : d0 + DH].T.astype(np.float16)
        xt = np.zeros((2 * DH, XPITCH), dtype=np.float16)
        xt[:DH, 0:XROW] = xp[:, 0:XROW]
        xt[DH:, 0:XROW] = xp[:, C : C + XROW]
        # w rows for this shard, rearranged so partition p holds the K
        # weights of channel blk*128 + p at free cols [blk*K, blk*K + K)
        w_sh = w[d0 : d0 + DH].reshape(NBLK, 128, K)
        wt = (
            w_sh.transpose(1, 0, 2).reshape(128, NBLK * K).astype(np.float32)
        )
        in_maps.append(
            {
                "xt": np.ascontiguousarray(xt),
                "wt": np.ascontiguousarray(wt),
                "dg": dg,
            }
        )
    return in_maps


def kernel(x, w):
    x = np.asarray(x, dtype=np.float32)
    w = np.asarray(w, dtype=np.float32)
    assert x.shape == (B, L, D) and w.shape == (D, K)

    if "nc" not in _cache:
        _cache["nc"] = _build_bass()
    nc = _cache["nc"]

    in_maps = _shard_inputs(x, w)
    res = None
    for attempt in range(3):
        try:
            res = run_bass_kernel_spmd(nc, in_maps, core_ids=list(range(N_CORES)))
            break
        except Exception:
            if attempt == 2:
                raise
    _cache["last_results"] = res

    out = np.empty((B, L, D), dtype=np.float32)
    for core in range(N_CORES):
        b, half = divmod(core, 2)
        d0 = half * DH
        o3 = res.results[core]["ot"].reshape(2, DH, C)
        full = np.concatenate([o3[0], o3[1]], axis=1)  # [DH, L]
        out[b, :, d0 : d0 + DH] = full.T.astype(np.float32)
    return out


# revision 8
# speedup vs baseline: 1.1391x; 1.0156x over previous
"""Depthwise causal Conv1d (k=4) + SiLU on 8 Trainium2 NeuronCores.

Problem: x [4, 4096, 2048] f32, w [2048, 4] f32,
out[b, t, d] = silu(sum_j w[d, j] * x[b, t - 3 + j, d])   (zero-padded left).

Sharding: 8 cores = 4 batches x 2 channel-halves. Depthwise conv is
independent per channel, so channel sharding needs no halo exchange.

Layout: each core receives its shard host-transposed to [channels, time]
(channels on SBUF partitions); per-channel weights are per-partition
scalars and causal shifts are free-dim AP offsets. Both DRAM tensors are
HALF-MAJOR so every [128, ~2048] DMA row is dense: xt row (half*DH+ch)
holds x[ch, half*2048-3 : half*2048+2048] (3-col halo duplicated on the
host), ot row (half*DH+ch) holds out[ch, half*2048 : +2048].

Precision: x and the output are host-cast fp16 (halves HBM traffic both
ways); products and adds stay fp16 (PE accumulates fp32 in PSUM); SiLU
computes fp32-internally on ACT. End-to-end relative error ~5e-4.

v5 design, tuned against NTFF profiles of v1-v4:
 - Per-core budget is DMA: ~16.9 MB at ~435 GB/s (R+W combined) ~ 40us;
   dense rows keep both directions near the fabric cap.
 - Work is cut into [128ch, 2048t] chunks. 5 blocks ride the
   TensorEngine as one back-to-back stream (p-states only reach 2.4 GHz
   under continuous execution): diag(w_j) matmuls, taps outer, 4-tap
   PSUM accumulation, SiLU straight out of PSUM. 3 blocks ride DVE:
   4 shift-rebased tensor_scalar products + pair-packed adds.
 - Diag stationaries are built on device from a [128,128] identity mask
   (20 cheap DVE ops) instead of a 1 MB HBM tensor.
 - ACT does only SiLU; its emission order is hand-matched to production
   order so the in-order engine never blocks a ready PSUM drain behind
   a not-yet-ready DVE silu. Stores follow silus chunk by chunk, so the
   store stream starts ~5us in and overlaps the load stream.
 - Loads on SyncE (HWDGE, 8-chunk runway), stores on GpSimd (SWDGE).
"""

import sys
import types

import numpy as np

import concourse.bass as bass
import concourse.bacc as bacc
import concourse.mybir as mybir
from concourse.tile import TileContext
from concourse.bass_utils import run_bass_kernel_spmd


def _ensure_ntff_hook():
    """bass_utils imports antenv.axon_hooks when BASS_TRACE is set; that
    module is absent on this image. Install a shim so tracing works when
    possible and degrades gracefully (instead of crashing) when not."""
    try:
        import antenv.axon_hooks  # noqa: F401

        return
    except ImportError:
        pass
    try:
        import antenv

        hook = None
        try:
            if "/root/.axon_site" not in sys.path:
                sys.path.insert(0, "/root/.axon_site")
            from trn_agent_boot.trn_boot import _ntff_profile_via_ctypes

            hook = _ntff_profile_via_ctypes("/opt/axon/libaxon_pjrt.so")
        except Exception:
            hook = None
        mod = types.ModuleType("antenv.axon_hooks")
        mod._hook = hook
        mod.get_axon_ntff_profile_hook = lambda: mod._hook
        mod.set_axon_ntff_profile_hook = lambda h: setattr(mod, "_hook", h)
        sys.modules["antenv.axon_hooks"] = mod
        antenv.axon_hooks = mod
    except Exception:
        pass


_ensure_ntff_hook()

B, L, D = 4, 4096, 2048
K = 4
PAD = K - 1
N_CORES = 8
DH = D // 2            # channels per core
NBLK = DH // 128       # 128-partition channel blocks per core
C = 2048               # time chunk (half of L)
XROW = C + PAD         # 2051 data cols per xt row
XPITCH = 2064          # xt row pitch (fp16 elems), 32B-aligned

MID_DT = mybir.dt.float16
PE_BLKS = [1, 3, 5, 7, 6]   # blocks on the TensorEngine, in stream order
DVE_BLKS = [0, 2, 4]        # blocks on DVE
_PE_IDX = {b: i for i, b in enumerate(PE_BLKS)}

_cache = {}


def _build_bass():
    nc = bacc.Bacc()
    # half-major inputs/outputs: row (half*DH + ch)
    xt = nc.dram_tensor("xt", [2 * DH, XPITCH], MID_DT, kind="ExternalInput")
    wt = nc.dram_tensor("wt", [128, NBLK * K], mybir.dt.float32, kind="ExternalInput")
    # [128,128] identity mask; diag(w_j) stationaries are built on device
    dg = nc.dram_tensor("dg", [128, 128], MID_DT, kind="ExternalInput")
    ot = nc.dram_tensor("ot", [2 * DH, C], MID_DT, kind="ExternalOutput")
    f32 = mybir.dt.float32

    with TileContext(nc) as tc:
        with tc.tile_pool(name="pool", bufs=2) as pool, \
             tc.tile_pool(name="psum", bufs=2, space="PSUM") as psum_pool:
            # Warmup: a tiny Silu forces the silu activation-table set to
            # load during the initial DMA wait; it is the only table load
            # in the whole kernel.
            warm = pool.tile([128, 2], MID_DT, tag="warm", bufs=1)
            nc.vector.memset(warm[:], 0.0)
            nc.scalar.activation(warm[:], warm[:], mybir.ActivationFunctionType.Silu)

            w = pool.tile([128, NBLK * K], f32, tag="w", bufs=1)
            nc.sync.dma_start(out=w[:], in_=wt[:, :])
            mask = pool.tile([128, 128], MID_DT, tag="mask", bufs=1)
            nc.sync.dma_start(out=mask[:], in_=dg[:, :])

            # diag(w[blk*128+p, j]) stationaries: [128,128] per-partition
            # scalar muls of the identity mask, first PE block first. They
            # run on ACT (activation Copy with per-partition scale), which
            # is otherwise idle until the first PSUM drain ~6us in.
            dgw = pool.tile([128, len(PE_BLKS) * K * 128], MID_DT, tag="dgw", bufs=1)
            for blk in PE_BLKS:
                bi = _PE_IDX[blk]
                for j in range(K):
                    c0 = (bi * K + j) * 128
                    nc.scalar.mul(
                        dgw[:, c0 : c0 + 128],
                        mask[:],
                        w[:, blk * K + j : blk * K + j + 1],
                    )

            def wj(blk, j):
                return w[:, blk * K + j : blk * K + j + 1]

            xts = {}

            def load(blk, half):
                r0 = half * DH + blk * 128
                x = pool.tile([128, XROW + 1], MID_DT, tag="x", bufs=8)
                nc.sync.dma_start(out=x[:, 0:XROW], in_=xt[r0 : r0 + 128, 0:XROW])
                xts[(blk, half)] = x

            ps_of = {}

            def pe_mm(blk, half, off=0, tl=C):
                """Fill one [128, tl] PSUM tile with the 4-tap conv of one
                (sub)chunk. Taps outer: one stationary per 4 matmuls."""
                bi = _PE_IDX[blk]
                x = xts[(blk, half)]
                ps = psum_pool.tile([128, tl], f32, tag="ps", bufs=2)
                ps_of[(blk, half, off)] = ps
                for j in range(K):
                    lw = dgw[:, (bi * K + j) * 128 : (bi * K + j + 1) * 128]
                    for c in range(tl // 512):
                        h0 = off + c * 512 + j
                        nc.tensor.matmul(
                            ps[:, c * 512 : (c + 1) * 512],
                            lw,
                            x[:, h0 : h0 + 512],
                            start=(j == 0),
                            stop=(j == K - 1),
                        )

            qe_of = {}

            def dve_chain(blk, half, off=0, tl=C):
                """Elementwise (sub)chunk: 4 shift-rebased products, pair-
                packed adds (qe=[q0|q2] + qo=[q1|q3], then fold into qe0)."""
                x = xts[(blk, half)]
                qe = pool.tile([128, 2, tl], MID_DT, tag="qe", bufs=2)
                qo = pool.tile([128, 2, tl], MID_DT, tag="qo", bufs=2)
                qe_of[(blk, half, off)] = qe
                o = off
                nc.vector.tensor_scalar_mul(qe[:, 0, :], x[:, o : o + tl], wj(blk, 0))
                nc.vector.tensor_scalar_mul(
                    qo[:, 0, :], x[:, o + 1 : o + 1 + tl], wj(blk, 1))
                nc.vector.tensor_scalar_mul(
                    qe[:, 1, :], x[:, o + 2 : o + 2 + tl], wj(blk, 2))
                nc.vector.tensor_scalar_mul(
                    qo[:, 1, :], x[:, o + 3 : o + 3 + tl], wj(blk, 3))
                nc.vector.tensor_add(qe[:, :, :], qe[:, :, :], qo[:, :, :])
                nc.vector.tensor_add(qe[:, 0, :], qe[:, 0, :], qe[:, 1, :])

            def fin(blk, half, off=0, tl=C):
                """SiLU (PSUM for PE chunks, SBUF for DVE chunks) + dense
                store of the finished [128, tl] chunk."""
                r0 = half * DH + blk * 128
                o = pool.tile([128, tl], MID_DT, tag="o", bufs=6)
                if (blk, half, off) in ps_of:
                    src = ps_of.pop((blk, half, off))[:]
                else:
                    src = qe_of.pop((blk, half, off))[:, 0, :]
                nc.scalar.activation(o[:], src, mybir.ActivationFunctionType.Silu)
                nc.gpsimd.dma_start(out=ot[r0 : r0 + 128, off : off + tl], in_=o[:])

            # Chunk schedule. PE stream: block 6 pulled early, the last
            # chunk (7,1) split at 1024 so the drain is fine-grained; same
            # for the last DVE chunk (4,1). Loads are emitted in need-time
            # order (the bufs=8 x-pool gives an 8-chunk runway); compute is
            # emitted per-engine back-to-back; ACT/store order approximates
            # production order.
            P = [(1, 0, 0, C), (6, 0, 0, C), (3, 0, 0, C), (6, 1, 0, C),
                 (5, 0, 0, C), (7, 0, 0, C), (1, 1, 0, C), (3, 1, 0, C),
                 (5, 1, 0, C), (7, 1, 0, 1024), (7, 1, 1024, 1024)]
            E = [(0, 0, 0, C), (2, 0, 0, C), (4, 0, 0, C), (0, 1, 0, C),
                 (2, 1, 0, C), (4, 1, 0, 1024), (4, 1, 1024, 1024)]
            load_order = [
                (1, 0), (0, 0), (6, 0), (3, 0), (2, 0), (6, 1), (4, 0),
                (5, 0), (7, 0), (0, 1), (1, 1), (3, 1), (2, 1), (5, 1),
                (4, 1), (7, 1),
            ]
            for blk, half in load_order:
                load(blk, half)

            # Per-engine compute streams (Tile resolves cross-engine deps;
            # program order only pins intra-engine order).
            for ch in P:
                pe_mm(*ch)
            for ch in E:
                dve_chain(*ch)

            # ACT + store order matched to expected completion times:
            # PE chunks ~3.9us apart, DVE chunks ~6.4us apart.
            fin_order = [
                P[0], P[1], E[0], P[2], E[1], P[3], P[4], E[2], P[5],
                P[6], E[3], P[7], E[4], P[8], E[5], P[9], E[6], P[10],
            ]
            for ch in fin_order:
                fin(*ch)
    nc.compile()
    return nc


def _shard_inputs(x, w):
    in_maps = []
    dg = np.eye(128, dtype=np.float16)
    for core in range(N_CORES):
        b, half = divmod(core, 2)
        d0 = half * DH
        xp = np.zeros((DH, PAD + L), dtype=np.float16)
        xp[:, PAD:] = x[b, :, d0 : d0 + DH].T.astype(np.float16)
        xt = np.zeros((2 * DH, XPITCH), dtype=np.float16)
        xt[:DH, 0:XROW] = xp[:, 0:XROW]
        xt[DH:, 0:XROW] = xp[:, C : C + XROW]
        # w rows for this shard, rearranged so partition p holds the K
        # weights of channel blk*128 + p at free cols [blk*K, blk*K + K)
        w_sh = w[d0 : d0 + DH].reshape(NBLK, 128, K)
        wt = (
            w_sh.transpose(1, 0, 2).reshape(128, NBLK * K).astype(np.float32)
        )
        in_maps.append(
            {
                "xt": np.ascontiguousarray(xt),
                "wt": np.ascontiguousarray(wt),
                "dg": dg,
            }
        )
    return in_maps


def kernel(x, w):
    x = np.asarray(x, dtype=np.float32)
    w = np.asarray(w, dtype=np.float32)
    assert x.shape == (B, L, D) and w.shape == (D, K)

    if "nc" not in _cache:
        _cache["nc"] = _build_bass()
    nc = _cache["nc"]

    in_maps = _shard_inputs(x, w)
    res = None
    for attempt in range(3):
        try:
            res = run_bass_kernel_spmd(nc, in_maps, core_ids=list(range(N_CORES)))
            break
        except Exception:
            if attempt == 2:
                raise
    _cache["last_results"] = res

    out = np.empty((B, L, D), dtype=np.float32)
    for core in range(N_CORES):
        b, half = divmod(core, 2)
        d0 = half * DH
        o3 = res.results[core]["ot"].reshape(2, DH, C)
        full = np.concatenate([o3[0], o3[1]], axis=1)  # [DH, L]
        out[b, :, d0 : d0 + DH] = full.T.astype(np.float32)
    return out
